# revision 32
# baseline (speedup 1.0000x reference)
"""Trainium2 Bass kernel for nn_CrossAttention (channel cross-attention block).

Per-sample computation (B=8 samples, one per NeuronCore, data-parallel).

Algebraic restructure (projection biases are zero for this problem,
asserted host-side):

  scores = (Wq q)(Wk v)^T / 96 = Wq (q v^T) Wk^T / 96
  out    = attn (Wv q)         = (attn Wv) q

leaving FOUR N-sized matmuls per sample:
  G = q v^T          [C,C]   (contraction over N = 9216)
  Z = (attn Wv) q    [C,N]
  conv1, conv2       [C,N]

Design notes (cost-model driven):
- G runs in fp8e4m3 with MatmulPerfMode.DoubleRow: 256-deep contraction per
  pass at 0.5 cycles/row -> 4x faster than bf16.  qT/vT are transposed and
  pair-interleaved ON THE HOST (free) and DMA'd as fp8: zero PE transposes.
- The Z/Y matmul also runs fp8 DoubleRow with an error-compensating 3-term
  split: q = qh + ql (host-quantized residual pair), 16*MT' = mh + ml
  (device-quantized off the PSUM).  Y ~ qh*mh + qh*ml + ql*mh keeps the
  error at bf16 level while running 2.7x faster than the bf16 version.
- q for Y is host-relaid k-major (qk[ch, kk*256+c'] = q[ch, c'*36+kk]) so
  the permute-fused stationary slices are contiguous and phase B can start
  before the whole tensor has landed.
- A 1-element warmup matmul at t~300ns starts the PE p-state ramp clock, so
  real matmuls run at full clock from ~3.3us.  A dummy Exp activation early
  on the Act queue absorbs the one-time activation-table load.
- Scores are computed TRANSPOSED (ST[d,c]) so attn^T comes straight from
  exp() with no PE transposes; softmax row-sums via an all-ones matmul, and
  the 1/sum normalization -- a per-column scale that commutes through the
  positive-homogeneous BN(bnt=0)/Lrelu/1x1-conv epilogue -- is fused into
  the output-copy tensor_tensor multiply.  (Needs bnt==bo1==bo2==0.)
- G accumulates in three parts (pairs 0-17/18-26/27-35), each feeding the
  scores matmul as soon as its copies land, overlapping the DMA tail.
"""
import numpy as np
import ml_dtypes

import concourse.bass as bass
import concourse.mybir as mybir
import concourse.tile as tile
from concourse.bass_utils import run_bass_kernel_spmd

B, C, HH, WW = 8, 256, 96, 96
N = HH * WW            # 9216
P = 128                # partitions
KB = 36                # n = c'*36 + kk   (9216 = 256*36)
NPAIR = 36             # DoubleRow chunk-pairs for G (9216 = 36*2*128)
f32 = mybir.dt.float32
bf16 = mybir.dt.bfloat16
fp8 = mybir.dt.float8e4
AF = mybir.ActivationFunctionType
ALU = mybir.AluOpType
DR = mybir.MatmulPerfMode.DoubleRow
ALPHA = 0.01           # LeakyReLU slope
GW = NPAIR * 2 * C     # 18432 columns of qt2/vt2 (and of qkh/qkl)
# G accumulation parts (pair lists).  Parts 1/2 interleave the SP/Pool
# vt2 sub-chunks so each part's matmuls start as soon as the FIRST
# sub-chunks land instead of bunching after the last one.
GPART = (tuple(range(0, 18)),
         tuple(range(18, 23)) + tuple(range(27, 32)),
         tuple(range(23, 27)) + tuple(range(32, 36)))

_cached = {}


def _build():
    nc = bass.Bass()
    act = nc.scalar      # Activation engine (+ HWDGE queue)
    dve = nc.vector      # DVE engine (no DMA queue)
    pool = nc.gpsimd     # Pool engine (SWDGE queue)
    sp = nc.sync         # SP (HWDGE queue)

    # host-packed tensors
    qkh_d = nc.dram_tensor("qkh", [P, GW], fp8, kind="ExternalInput")
    qkl_d = nc.dram_tensor("qkl", [P, GW], fp8, kind="ExternalInput")
    qt2_d = nc.dram_tensor("qt2", [P, GW], fp8, kind="ExternalInput")
    vt2_d = nc.dram_tensor("vt2", [P, GW], fp8, kind="ExternalInput")
    # w2 = [wqt(=Wq.T/96), wkt(=Wk.T)]; w3 = [16*wvn(=Wv), wo1t, wo2t]
    w2_d = nc.dram_tensor("w2", [2, C, C], bf16, kind="ExternalInput")
    w3_d = nc.dram_tensor("w3", [3, C, C], bf16, kind="ExternalInput")
    vec_d = nc.dram_tensor("vec1", [1, C], f32, kind="ExternalInput")
    out_d = nc.dram_tensor("out", [C, N], bf16, kind="ExternalOutput")

    with tile.TileContext(nc) as tc:
        with (
            tc.tile_pool(name="wpool", bufs=1) as wp,
            tc.tile_pool(name="gpool", bufs=1) as gp,
            tc.tile_pool(name="qpool", bufs=1) as qp,
            tc.tile_pool(name="spool", bufs=1) as sb,
        ):
            # ---- PE p-state warmup (ramp clock) ----
            ones = wp.tile([P, P], bf16, name="ones")
            dve.memset(ones[:], 1.0)
            with tc.tile_pool(name="ps_w", bufs=1, space="PSUM") as ps_w:
                pw = ps_w.tile([1, 1], f32, name="pw")
                nc.tensor.matmul(pw[:], ones[:, 0:1], ones[:, 0:1],
                                 start=True, stop=True)

            qt2 = gp.tile([P, GW], fp8, name="qt2")
            vt2 = gp.tile([P, GW], fp8, name="vt2")
            qkh = qp.tile([P, GW], fp8, name="qkh")
            qkl = qp.tile([P, GW], fp8, name="qkl")
            w2t = [wp.tile([P, 2 * C], bf16, name=f"w2_{i}") for i in range(2)]
            w3t = [wp.tile([P, 3 * C], bf16, name=f"w3_{i}") for i in range(2)]
            v1t = [wp.tile([P, 1], f32, name=f"v1_{i}") for i in range(2)]

            # ---------------- DMA schedule ----------------
            HG = GW // 2          # pairs 0-17
            QG = GW // 4          # 4608-col chunks of the G streams
            PR = 2 * C            # columns per pair

            def vslice(q_eng, lo, hi):
                q_eng.dma_start(vt2[:, lo * PR:hi * PR],
                                vt2_d[:, lo * PR:hi * PR])
            # SP: qT pairs 0-17 (2 chunks), vT pairs 18-26 (staggered),
            # small tensors, then qk-hi in kk-order chunks; out DMAs late.
            sp.dma_start(qt2[:, 0:QG], qt2_d[:, 0:QG])
            sp.dma_start(qt2[:, QG:HG], qt2_d[:, QG:HG])
            vslice(sp, 18, 23)
            vslice(sp, 23, 27)
            for i in range(2):
                sp.dma_start(
                    w3t[i][:].rearrange("p (w c) -> p w c", c=C),
                    w3_d[:, i * P:(i + 1) * P, :].rearrange("w p c -> p w c"))
                sp.dma_start(v1t[i][:],
                             vec_d[:, i * P:(i + 1) * P].rearrange("a b -> b a"))
            QKC = GW // 3
            for j in range(3):
                sp.dma_start(qkh[:, j * QKC:(j + 1) * QKC],
                             qkh_d[:, j * QKC:(j + 1) * QKC])
            # Act: vT pairs 0-17 (2 chunks) and wq/wk; then frees up for the
            # softmax-chain activation work.
            act.dma_start(vt2[:, 0:QG], vt2_d[:, 0:QG])
            act.dma_start(vt2[:, QG:HG], vt2_d[:, QG:HG])
            for i in range(2):
                act.dma_start(
                    w2t[i][:].rearrange("p (w c) -> p w c", c=C),
                    w2_d[:, i * P:(i + 1) * P, :].rearrange("w p c -> p w c"))
            dume = sb.tile([P, 1], bf16, name="dume")
            # Pool: qT pairs 18-35 (2 chunks), vT pairs 27-35, then qk-lo;
            # out DMAs (o2=1) late.
            pool.dma_start(qt2[:, HG:HG + QG], qt2_d[:, HG:HG + QG])
            pool.dma_start(qt2[:, HG + QG:GW], qt2_d[:, HG + QG:GW])
            vslice(pool, 27, 32)
            vslice(pool, 32, 36)
            for j in range(3):
                pool.dma_start(qkl[:, j * QKC:(j + 1) * QKC],
                               qkl_d[:, j * QKC:(j + 1) * QKC])

            w23 = [w2t[i][:].rearrange("p (w c) -> p w c", c=C) for i in range(2)]
            w33 = [w3t[i][:].rearrange("p (w c) -> p w c", c=C) for i in range(2)]
            wqt = [w23[i][:, 0, :] for i in range(2)]
            wkt = [w23[i][:, 1, :] for i in range(2)]
            wvn = [w33[i][:, 0, :] for i in range(2)]     # 16*Wv
            wo1t = [w33[i][:, 1, :] for i in range(2)]
            wo2t = [w33[i][:, 2, :] for i in range(2)]
            bns = [v1t[i][:, 0:1] for i in range(2)]      # gamma/sqrt(var)/16

            mh = sb.tile([P, 2 * C], fp8, name="mh")   # fp8(16*MT'), i-major
            ml = sb.tile([P, 2 * C], fp8, name="ml")   # fp8 residual
            rb2 = sb.tile([P, 512], f32, name="rb2")   # 1/rowsum, bcast+doubled

            # ============ Phase G: Gram matrix q v^T (fp8 DoubleRow) =======
            # G[cb*128+m, d] = sum_n q[cb*128+m, n] v[d, n], in three
            # accumulation parts so the scores matmul overlaps the DMA tail.
            # NOTE: PSUM start=True zeroes a full 2KB bank region, so every
            # accumulation group needs its own bank.
            qt4 = qt2[:].rearrange("k (pr i c) -> k pr i c", i=2, c=C)
            vt4 = vt2[:].rearrange("k (pr i c) -> k pr i c", i=2, c=C)
            g_sb = {}
            with tc.tile_pool(name="ps_1", bufs=1, space="PSUM") as ps_sa:
                ps1 = [ps_sa.tile([P, C], f32, name=f"ps1{eb}")
                       for eb in range(2)]

                def emit_ps1(part):
                    # S1T[e, c] += sum_g Gpart[g, e] WqT[g, c]
                    for eb in range(2):
                        for cb in range(2):
                            nc.tensor.matmul(
                                ps1[eb][:],
                                g_sb[(part, cb)][:, eb * P:(eb + 1) * P],
                                wqt[cb][:],
                                start=(part == 0 and cb == 0),
                                stop=(part == 2 and cb == 1),
                                skip_group_check=True)

                with tc.tile_pool(name="ps_g", bufs=1, space="PSUM") as ps_g:
                    psum_g = {(pt, cb): ps_g.tile([P, C], f32,
                                                  name=f"pg{pt}{cb}")
                              for pt in range(3) for cb in range(2)}

                    def emit_gpart(pt):
                        prs_ = GPART[pt]
                        for j, pr in enumerate(prs_):
                            for cb in range(2):
                                nc.tensor.matmul(
                                    psum_g[(pt, cb)][:],
                                    qt4[:, pr, :, cb * P:(cb + 1) * P],
                                    vt4[:, pr, :, :],
                                    start=(j == 0),
                                    stop=(j == len(prs_) - 1),
                                    perf_mode=DR, skip_group_check=True)

                    def emit_gcopy(pt):
                        for cb in range(2):
                            g = sb.tile([P, C], bf16, name=f"g{pt}{cb}")
                            if cb == 0:
                                act.activation(g[:], psum_g[(pt, cb)][:],
                                               AF.Identity)
                            else:
                                dve.tensor_copy(g[:], psum_g[(pt, cb)][:])
                            g_sb[(pt, cb)] = g

                    emit_gpart(0)
                    emit_gcopy(0)
                    # table-load absorber: the first activation on Act is
                    # charged the table load; burn it in the idle window,
                    # and make it Exp so the real exp doesn't reload.
                    act.activation(dume[:], ones[:, 0:1], AF.Exp)
                    emit_gpart(1)
                    emit_ps1(0)     # fills the PE idle before part-2 data
                    emit_gcopy(1)
                    emit_gpart(2)
                    emit_gcopy(2)
                    emit_ps1(1)
                    emit_ps1(2)

                # ==== Phase S tail: scores^T, exp, row-sums, M^T ====
                # ST[d, c] = sum_e WK[d, e] S1T[e, c] -> exp -> ET (= attn^T
                # unnormalized); row sums via all-ones matmul; the 1/sum
                # normalization is deferred to the output copy (commutes
                # through the per-column positive-homogeneous epilogue).
                with tc.tile_pool(name="ps_2", bufs=1, space="PSUM") as ps_sb:
                    s1t_sb = []
                    for eb in range(2):
                        s1 = sb.tile([P, C], bf16, name=f"s1t{eb}")
                        if eb == 0:
                            act.activation(s1[:], ps1[eb][:], AF.Identity)
                        else:
                            dve.tensor_copy(s1[:], ps1[eb][:])
                        s1t_sb.append(s1)
                    et_sb = []
                    pst = [ps_sb.tile([P, C], f32, name=f"pst{db}")
                           for db in range(2)]
                    for db in range(2):
                        for eb in range(2):
                            nc.tensor.matmul(pst[db][:],
                                             wkt[eb][:, db * P:(db + 1) * P],
                                             s1t_sb[eb][:], start=(eb == 0),
                                             stop=(eb == 1),
                                             skip_group_check=True)
                        # scores ~ N(0, ~0.13): exp cannot overflow
                        et = sb.tile([P, C], bf16, name=f"et{db}")
                        act.activation(et[:], pst[db][:], AF.Exp)
                        et_sb.append(et[:])
                    # 16*MT'[ib*128+ch, c] = sum_d 16*Wv[d, ch] ET[d, c],
                    # then split into fp8 hi+lo for the DoubleRow Y matmul.
                    pmt = [ps_sb.tile([P, C], f32, name=f"pmt{ib}")
                           for ib in range(2)]
                    for ib in range(2):
                        for db in range(2):
                            nc.tensor.matmul(pmt[ib][:],
                                             wvn[db][:, ib * P:(ib + 1) * P],
                                             et_sb[db], start=(db == 0),
                                             stop=(db == 1),
                                             skip_group_check=True)
                        act.activation(mh[:, ib * C:(ib + 1) * C],
                                       pmt[ib][:], AF.Identity)
                        dve.scalar_tensor_tensor(
                            ml[:, ib * C:(ib + 1) * C],
                            mh[:, ib * C:(ib + 1) * C], -1.0, pmt[ib][:],
                            op0=ALU.mult, op1=ALU.add)
                    # row sums rs[c] = sum_d ET[d, c], broadcast to all
                    # partitions via an all-ones stationary; rb2 = [1/rs|1/rs]
                    prs = ps_sb.tile([P, C], f32, name="prs")
                    for db in range(2):
                        nc.tensor.matmul(prs[:], ones[:], et_sb[db],
                                         start=(db == 0), stop=(db == 1),
                                         skip_group_check=True)
                    dve.reciprocal(rb2[:, 0:C], prs[:])
                    dve.tensor_copy(rb2[:, C:2 * C], rb2[:, 0:C])

            # ========== Phase B: Y (fused permute) -> conv1 -> conv2 ======
            # 16*Y'[c', k*256+c] = sum_ch q[ch, c'*36+k] 16*MT'[ch, c]
            # via DoubleRow fp8 with 3-term hi/lo error compensation;
            # stationary = qk*[ (ch%128), k, ch//128, cp*128+c' ] contiguous.
            qkh4 = qkh[:].rearrange("k (kk i c) -> k kk i c", i=2, c=C)
            qkl4 = qkl[:].rearrange("k (kk i c) -> k kk i c", i=2, c=C)
            mh3 = mh[:].rearrange("k (i c) -> k i c", i=2)
            ml3 = ml[:].rearrange("k (i c) -> k i c", i=2)
            with (
                tc.tile_pool(name="yb", bufs=3) as yb,
                tc.tile_pool(name="ps_b", bufs=2, space="PSUM") as psb,
            ):
                LASTKP = N // 512 - 1

                def emit_h(kp, ys):
                    hs = []
                    for ob in range(2):
                        ph = psb.tile([P, 512], f32, name="ph", tag=f"ph{ob}",
                                      bufs=1)
                        nc.tensor.matmul(ph[:], wo1t[0][:, ob * P:(ob + 1) * P],
                                         ys[0][:], start=True, stop=False)
                        nc.tensor.matmul(ph[:], wo1t[1][:, ob * P:(ob + 1) * P],
                                         ys[1][:], start=False, stop=True)
                        h = yb.tile([P, 512], bf16, name="h", tag=f"h{ob}")
                        # bo1 == 0 (asserted host-side): Lrelu = (x*a) max x.
                        # An stt may read only ONE input from PSUM (and HW
                        # has no Pool stt), so h1 alternates: Act activation
                        # on even kps, DVE copy+stt on odd ones -- keeping
                        # both engines under the PE pace on average.
                        if ob == 0 or kp == LASTKP or kp % 2 == 0:
                            act.activation(h[:], ph[:], AF.Lrelu, alpha=ALPHA)
                        else:
                            t1 = yb.tile([P, 512], bf16, name="t1", tag="t1")
                            dve.tensor_copy(t1[:], ph[:])
                            dve.scalar_tensor_tensor(h[:], t1[:], ALPHA,
                                                     t1[:], op0=ALU.mult,
                                                     op1=ALU.max)
                        hs.append(h)
                    return hs

                def emit_f(kp, hs):
                    for o2 in range(2):
                        pf = psb.tile([P, 512], f32, name="pf", tag=f"pf{o2}",
                                      bufs=1)
                        nc.tensor.matmul(pf[:], wo2t[0][:, o2 * P:(o2 + 1) * P],
                                         hs[0][:], start=True, stop=False)
                        nc.tensor.matmul(pf[:], wo2t[1][:, o2 * P:(o2 + 1) * P],
                                         hs[1][:], start=False, stop=True)
                        ob_t = yb.tile([P, 512], bf16, name="ob", tag=f"ob{o2}")
                        # bo2 == 0: output copy fused with the deferred
                        # softmax normalization (per-column 1/rowsum)
                        dve.tensor_tensor(ob_t[:], pf[:], rb2[:], op=ALU.mult)
                        q_eng = sp if o2 == 0 else pool
                        q_eng.dma_start(out_d[o2 * P:(o2 + 1) * P,
                                              kp * 512:(kp + 1) * 512], ob_t[:])

                pend_y = []  # (kp, ys) awaiting conv1
                pend_h = []  # (kp, hs) awaiting conv2
                for kp in range(N // 512):
                    ys = []
                    for cp in range(2):
                        py_ = psb.tile([P, 512], f32, name="py", tag=f"py{cp}")
                        terms = ((qkh4, mh3), (qkh4, ml3), (qkl4, mh3))
                        for ki in range(2):
                            k = 2 * kp + ki
                            for t, (qx, mx) in enumerate(terms):
                                nc.tensor.matmul(
                                    py_[:, ki * C:(ki + 1) * C],
                                    qx[:, k, :, cp * P:(cp + 1) * P],
                                    mx[:],
                                    start=(t == 0), stop=(t == 2),
                                    perf_mode=DR)
                        y = yb.tile([P, 512], bf16, name="y", tag=f"y{cp}")
                        # bnt == 0 (asserted); bns/16 applied as act scale
                        act.activation(y[:], py_[:], AF.Lrelu,
                                       scale=bns[cp][:], alpha=ALPHA)
                        ys.append(y)
                    pend_y.append((kp, ys))
                    if len(pend_y) > 1:
                        kp1, ys1 = pend_y.pop(0)
                        pend_h.append((kp1, emit_h(kp1, ys1)))
                    if len(pend_h) > 1:
                        kp2, hs2 = pend_h.pop(0)
                        emit_f(kp2, hs2)
                while pend_y or pend_h:
                    if pend_h:
                        kp2, hs2 = pend_h.pop(0)
                        emit_f(kp2, hs2)
                    if pend_y:
                        kp1, ys1 = pend_y.pop(0)
                        pend_h.append((kp1, emit_h(kp1, ys1)))
    return nc


def _split_waits(nc):
    """Walrus's per-instruction ISA structs carry a single sem-wait slot and
    it refuses instructions with more ("Too many sync wait commands").  Tile
    freely attaches several.  Hoist all but one wait onto single-wait NoOps
    executed immediately before, on the same engine stream."""
    for f in nc.m.functions:
        for bb in f.blocks:
            new = []
            for inst in bb.instructions:
                si = inst.sync_info
                if (si is not None and si.on_wait and len(si.on_wait) > 1
                        and not isinstance(inst, (mybir.InstNoOp,
                                                  mybir.InstEventSemaphore))):
                    for wi, w in enumerate(si.on_wait[:-1]):
                        new.append(mybir.InstNoOp(
                            name=f"{inst.name}-ws{wi}",
                            ins=[], outs=[],
                            engine=inst.engine,
                            sync_info=mybir.SyncInfo(on_wait=[w], on_update=[]),
                            bass_nofuse=True,
                        ))
                    inst.sync_info = mybir.SyncInfo(on_wait=[si.on_wait[-1]],
                                                    on_update=list(si.on_update))
                new.append(inst)
            bb.instructions[:] = new


def _prep(inputs):
    """Host-side prep: fold scales, transpose/re-lay data, cast dtypes."""
    f = np.float32
    bb = ml_dtypes.bfloat16
    f8 = ml_dtypes.float8_e4m3
    # this kernel specializes the projection/conv biases (and the BN shift)
    # to zero -- true for this problem's inputs; the algebra, the epilogue
    # ops, and the deferred softmax normalization rely on it
    for b in ("bq", "bk", "bv", "bo1", "bo2"):
        assert not np.any(np.asarray(inputs[b])), f"nonzero {b} unsupported"
    bns = (inputs["bn_gamma"].astype(f)
           / np.sqrt(inputs["bn_var"].astype(f) + np.float32(1e-4))).astype(f)
    bnt = (inputs["bn_beta"].astype(f)
           - inputs["bn_mean"].astype(f) * bns).astype(f)
    assert not np.any(bnt), "nonzero BN shift unsupported"
    scale = f(1.0) / f(np.sqrt(N))
    wqt = (inputs["Wq"].T.astype(f) * scale).astype(bb)
    wkt = inputs["Wk"].T.astype(f).astype(bb)
    wvn16 = (inputs["Wv"].astype(f) * f(16.0)).astype(bb)
    wo1t = inputs["Wo1"].T.astype(f).astype(bb)
    wo2t = inputs["Wo2"].T.astype(f).astype(bb)
    w2 = np.ascontiguousarray(np.stack([wqt, wkt], axis=0))
    w3 = np.ascontiguousarray(np.stack([wvn16, wo1t, wo2t], axis=0))
    common = {
        "w2": w2, "w3": w3,
        "vec1": np.ascontiguousarray((bns / f(16.0)).reshape(1, C)),
    }
    q = np.asarray(inputs["q"], dtype=f).reshape(B, C, N)
    v = np.asarray(inputs["v"], dtype=f).reshape(B, C, N)
    in_maps = []
    for b in range(B):
        m = dict(common)
        qb = q[b]
        qh = qb.astype(f8)
        ql = (qb - qh.astype(f)).astype(f8)
        # qk*[k, kk*512 + i*256 + c'] = q*[i*128+k, c'*36+kk]
        for nm, arr in (("qkh", qh), ("qkl", ql)):
            qk = (arr.reshape(2, P, C, KB).transpose(1, 3, 0, 2)
                  .reshape(P, GW))
            m[nm] = np.ascontiguousarray(qk)
        # qt2[k, pr*512 + i*256 + c] = q[c, (2*pr+i)*128 + k]
        qt2 = qb.reshape(C, NPAIR, 2, P).transpose(3, 1, 2, 0).reshape(P, GW)
        m["qt2"] = np.ascontiguousarray(qt2.astype(f8))
        vt2 = v[b].reshape(C, NPAIR, 2, P).transpose(3, 1, 2, 0).reshape(P, GW)
        m["vt2"] = np.ascontiguousarray(vt2.astype(f8))
        in_maps.append(m)
    return in_maps


def kernel(_trace=False, **inputs):
    if "nc" not in _cached:
        nc = _build()
        _split_waits(nc)
        _cached["nc"] = nc
    nc = _cached["nc"]
    in_maps = _prep(inputs)
    res = run_bass_kernel_spmd(nc, in_maps, core_ids=list(range(B)),
                               trace=_trace)
    out = np.stack([np.asarray(res.results[b]["out"]).astype(np.float32)
                    for b in range(B)], axis=0)
    if _trace:
        kernel.last_results = res
    return out.reshape(B, C, HH, WW)
